# revision 22
# baseline (speedup 1.0000x reference)
"""AttentionPairBias Trainium2 kernel (v2, j-major).

Sharding: split the 1024 query rows (i) across 8 cores. Every core computes
full k/v from the replicated s, its own q/g rows, the pair-bias for its
(i, j) block, attention + gated output projection for its rows. No
collectives; the host concatenates the row blocks.

v2 layout: z is host-transposed to [c, jc, i, jn] so each DMA line is 4KB
contiguous per partition. Pair-bias and attention run j-major: scores are
computed as S^T[j, i] per (head, j-chunk), the softmax denominator comes from
a ones-column appended to v, and LN(z) stats (mean via an extra wz column,
sum-of-squares via a per-i ones matmul on z^2) are applied in dense
[j, h, i] tiles. Squares run on the vector engine in bf16.
"""

import numpy as np
import ml_dtypes
from contextlib import ExitStack

import concourse.bass as bass
import concourse.mybir as mybir
import concourse.tile as tile
from concourse import bacc
from concourse.bass_utils import run_bass_kernel_spmd
from concourse.masks import make_identity

P = 128
N = 1024
C = 768
CC = C // P          # 6 chunks of the c_s contraction
CZ = 128             # pair channel dim
H = 16
HD = 48
NI = N // 8          # query rows per core
EPS = 1e-5
IG = 16              # i's per z processing group
F32 = mybir.dt.float32
F32R = mybir.dt.float32r
BF16 = mybir.dt.bfloat16
AF = mybir.ActivationFunctionType
OP = mybir.AluOpType


def _bcast(ap, parts=P):
    """Partition-broadcast view of a DRAM AP (step 0 over partitions)."""
    return bass.AP(tensor=ap.tensor, offset=ap.offset, ap=[[0, parts]] + list(ap.ap))


def build_kernel():
    nc = bacc.Bacc(None, target_bir_lowering=False)

    zt_d = nc.dram_tensor("zt", [CZ, 8, NI, P], BF16, kind="ExternalInput")
    s_d = nc.dram_tensor("s", [N, C], F32, kind="ExternalInput")
    smy_d = nc.dram_tensor("smy", [NI, C], F32, kind="ExternalInput")
    wq_d = nc.dram_tensor("wq", [C, 1024], BF16, kind="ExternalInput")  # [c, o-pad]
    wk_d = nc.dram_tensor("wk", [C, 1024], BF16, kind="ExternalInput")
    wv_d = nc.dram_tensor("wv", [C, C], BF16, kind="ExternalInput")
    wg_d = nc.dram_tensor("wg", [C, C], BF16, kind="ExternalInput")
    wo_d = nc.dram_tensor("wo", [C, C], BF16, kind="ExternalInput")   # Wo^T
    bq_d = nc.dram_tensor("bq", [8, P], F32, kind="ExternalInput")
    bk_d = nc.dram_tensor("bk", [8, P], F32, kind="ExternalInput")
    bv_d = nc.dram_tensor("bv", [C], F32, kind="ExternalInput")
    bg_d = nc.dram_tensor("bg", [C], F32, kind="ExternalInput")
    wz_d = nc.dram_tensor("wz", [CZ, 17], BF16, kind="ExternalInput")
    srow_d = nc.dram_tensor("srow", [H], BF16, kind="ExternalInput")
    beff_d = nc.dram_tensor("beff", [P, 8, H], BF16, kind="ExternalInput")  # [jn, jc, h]
    out_d = nc.dram_tensor("out", [NI, C], F32, kind="ExternalOutput")

    with tile.TileContext(nc) as tc, ExitStack() as ctx:
        const = ctx.enter_context(tc.tile_pool(name="const", bufs=1))
        persist = ctx.enter_context(tc.tile_pool(name="persist", bufs=1))

        ident_bf = const.tile([P, P], BF16)
        make_identity(nc, ident_bf)
        eps_t = const.tile([P, 1], F32)
        nc.vector.memset(eps_t, EPS)
        ones_inv = const.tile([CZ, 1], BF16)
        nc.vector.memset(ones_inv, 1.0 / CZ)
        ones_invf = const.tile([CZ, 1], F32)
        nc.vector.memset(ones_invf, 1.0 / CZ)
        wz_sb = const.tile([CZ, 17], BF16)
        nc.sync.dma_start(wz_sb, wz_d[:])
        srow_bc = const.tile([P, H], BF16)
        nc.gpsimd.dma_start(srow_bc, _bcast(srow_d[:]))
        bv_bc = const.tile([P, C], F32)
        nc.gpsimd.dma_start(bv_bc, _bcast(bv_d[:]))
        bg_bc = const.tile([P, C], F32)
        nc.gpsimd.dma_start(bg_bc, _bcast(bg_d[:]))
        bq_sb = const.tile([P, 8], F32)
        nc.sync.dma_start(bq_sb, bq_d[:].rearrange("b p -> p b"))
        bk_sb = const.tile([P, 8], F32)
        nc.sync.dma_start(bk_sb, bk_d[:].rearrange("b p -> p b"))
        beff_sb = const.tile([P, 8, H], BF16)
        nc.sync.dma_start(beff_sb, beff_d[:])

        # Persistent activations
        kT = persist.tile([P, 8, N], BF16)       # k^T, 2 heads per 128-part block
        qT = persist.tile([P, 8, NI], BF16)
        v_aug = persist.tile([P, 8, H, HD + 1], BF16)  # [jn, jc, h, d|1]
        g_sb = persist.tile([P, C], F32)
        zb_all = persist.tile([P, 8, H, NI], BF16)     # [jn, jc, h, i]
        o_sb = persist.tile([P, C], F32)

        nc.vector.memset(v_aug[:, :, :, HD], 1.0)

        # ---------------- phase 1: LN(s) + projections ----------------
        with (
            tc.tile_pool(name="projw", bufs=1) as projw,
            tc.tile_pool(name="wpool", bufs=2) as wpool,
            tc.tile_pool(name="projp", bufs=2) as projp,
            tc.tile_pool(name="ppsum", bufs=2, space="PSUM") as ppsum,
        ):
            shatT = projw.tile([P, CC, N], BF16)     # LN(s)^T (no affine)
            shat_myT = projw.tile([P, CC, NI], BF16)

            def ln_rows(src_ap, n_rows_tiles, dstT):
                for r in range(n_rows_tiles):
                    s_t = projp.tile([P, C], F32, tag="s_t")
                    nc.sync.dma_start(s_t, src_ap[r * P:(r + 1) * P, :])
                    stats = projp.tile([P, 2, 6], F32, tag="stats")
                    s_win = s_t.rearrange("p (w f) -> p w f", w=2)
                    for w in range(2):
                        nc.vector.bn_stats(out=stats[:, w, :], in_=s_win[:, w, :])
                    mv = projp.tile([P, 2], F32, tag="mv")
                    nc.vector.bn_aggr(out=mv, in_=stats)
                    rstd = projp.tile([P, 1], F32, tag="rstd")
                    nc.scalar.activation(out=rstd, in_=mv[:, 1:2],
                                         func=AF.Sqrt, bias=eps_t)
                    nc.vector.reciprocal(rstd, rstd)
                    shat_t = projp.tile([P, C], BF16, tag="shat_t")
                    nc.vector.tensor_scalar(
                        out=shat_t, in0=s_t, scalar1=mv[:, 0:1], scalar2=rstd,
                        op0=OP.subtract, op1=OP.mult)
                    for cc in range(CC):
                        pst = ppsum.tile([P, P], BF16, tag="sm", name="pst")
                        nc.tensor.transpose(
                            pst, shat_t[:, cc * P:(cc + 1) * P], ident_bf)
                        nc.any.tensor_copy(
                            out=dstT[:, cc, r * P:(r + 1) * P], in_=pst)

            ln_rows(s_d[:], 8, shatT)
            ln_rows(smy_d[:], 1, shat_myT)

            # ---- projection emission closures (interleaved into z loop) ----
            def dma_w(dram, name, cols=None):
                w_sb = wpool.tile([P, CC, 1024], BF16, tag="w", name=name)
                dst = w_sb if cols is None else w_sb[:, :, :cols]
                nc.sync.dma_start(dst, dram[:].rearrange("(cc p) o -> p cc o", p=P))
                return w_sb

            wtiles = {}

            def blk_k(b, nh):
                pk = ppsum.tile([P, 512], F32, tag="big", name="pk")
                for cc in range(CC):
                    nc.tensor.matmul(
                        pk,
                        lhsT=wtiles["wk"][:, cc, b * P:(b + 1) * P],
                        rhs=shatT[:, cc, nh * 512:(nh + 1) * 512],
                        start=(cc == 0), stop=(cc == CC - 1))
                nc.scalar.activation(
                    out=kT[:, b, nh * 512:(nh + 1) * 512], in_=pk,
                    func=AF.Identity, bias=bk_sb[:, b:b + 1])

            def blk_q(b):
                pq_full = ppsum.tile([P, 512], F32, tag="big", name="pq")
                pq = pq_full[:, :NI]
                for cc in range(CC):
                    nc.tensor.matmul(
                        pq, lhsT=wtiles["wq"][:, cc, b * P:(b + 1) * P],
                        rhs=shat_myT[:, cc, :],
                        start=(cc == 0), stop=(cc == CC - 1))
                nc.scalar.activation(
                    out=qT[:, b, :], in_=pq,
                    func=AF.Identity, bias=bq_sb[:, b:b + 1])

            def blk_v(jo, nh):
                w = 512 if nh == 0 else 256
                pv_full = ppsum.tile([P, 512], F32, tag="big", name="pv_full")
                pv = pv_full[:, :w]
                for cc in range(CC):
                    nc.tensor.matmul(
                        pv,
                        lhsT=shatT[:, cc, jo * P:(jo + 1) * P],
                        rhs=wtiles["wv"][:, cc, nh * 512:nh * 512 + w],
                        start=(cc == 0), stop=(cc == CC - 1))
                nc.vector.tensor_tensor(
                    pv, pv, bv_bc[:, nh * 512:nh * 512 + w], OP.add)
                if nh == 0:
                    nc.any.tensor_copy(
                        out=v_aug[:, jo, 0:10, 0:HD],
                        in_=pv[:, 0:480].rearrange("p (h d) -> p h d", h=10))
                    nc.any.tensor_copy(
                        out=v_aug[:, jo, 10, 0:32], in_=pv[:, 480:512])
                else:
                    nc.any.tensor_copy(
                        out=v_aug[:, jo, 10, 32:HD], in_=pv[:, 0:16])
                    nc.any.tensor_copy(
                        out=v_aug[:, jo, 11:16, 0:HD],
                        in_=pv[:, 16:256].rearrange("p (h d) -> p h d", h=5))

            def blk_g(nh):
                w = 512 if nh == 0 else 256
                pg_full = ppsum.tile([P, 512], F32, tag="big", name="pg_full")
                pg = pg_full[:, :w]
                for cc in range(CC):
                    nc.tensor.matmul(
                        pg,
                        lhsT=shat_myT[:, cc, :],
                        rhs=wtiles["wg"][:, cc, nh * 512:nh * 512 + w],
                        start=(cc == 0), stop=(cc == CC - 1))
                nc.vector.tensor_tensor(
                    pg, pg, bg_bc[:, nh * 512:nh * 512 + w], OP.add)
                nc.scalar.activation(
                    out=g_sb[:, nh * 512:nh * 512 + w], in_=pg, func=AF.Sigmoid)

            # schedule: weight DMA emitted at first block of its consumer run
            sched = {
                0: [("dma", "wk", wk_d, None)] + [("k", b, nh) for b in range(2) for nh in range(2)] + [("k", 2, 0)],
                1: [("k", 2, 1)] + [("k", b, nh) for b in range(3, 5) for nh in range(2)],
                2: [("k", b, nh) for b in range(5, 8) for nh in range(2)],
                3: [("dma", "wq", wq_d, None), ("dma", "wv", wv_d, C)] + [("q", b) for b in range(8)],
                4: [("v", jo, nh) for jo in range(2) for nh in range(2)] + [("v", 2, 0)],
                5: [("v", 2, 1)] + [("v", jo, nh) for jo in range(3, 5) for nh in range(2)],
                6: [("dma", "wg", wg_d, C)] + [("v", jo, nh) for jo in range(5, 8) for nh in range(2)],
                7: [("g", 0), ("g", 1)],
            }

            def run_blocks(items):
                for it in items:
                    if it[0] == "dma":
                        wtiles[it[1]] = dma_w(it[2], it[1], it[3])
                    elif it[0] == "k":
                        blk_k(it[1], it[2])
                    elif it[0] == "q":
                        blk_q(it[1])
                    elif it[0] == "v":
                        blk_v(it[1], it[2])
                    elif it[0] == "g":
                        blk_g(it[1])

            # pair bias from z (j-major), projections woven in per jc
            gidx = 0
            for jc in range(8):
                for ig in range(NI // IG):
                    zt_sb = zp.tile([CZ, IG, P], BF16, tag="zt")
                    nc.sync.dma_start(
                        zt_sb, zt_d[:, jc, ig * IG:(ig + 1) * IG, :])
                    if gidx % 2 == 0:
                        zsq = zqf.tile([CZ, IG, P], F32, tag="zsqf")
                        ones_use = ones_invf
                        nc.gpsimd.tensor_tensor(zsq, zt_sb, zt_sb, OP.mult)
                    elif gidx % 4 == 1:
                        zsq = zq.tile([CZ, IG, P], BF16, tag="zsq")
                        ones_use = ones_inv
                        nc.scalar.activation(out=zsq, in_=zt_sb, func=AF.Square)
                    else:
                        zsq = zq.tile([CZ, IG, P], BF16, tag="zsq")
                        ones_use = ones_inv
                        nc.vector.tensor_tensor(zsq, zt_sb, zt_sb, OP.mult)
                    gidx += 1
                    pz = zpsum.tile([P, 18, IG], F32, tag="pz")
                    for ii in range(IG):
                        nc.tensor.matmul(
                            pz[:, 0:17, ii], lhsT=zt_sb[:, ii, :],
                            rhs=wz_sb, start=True, stop=True)
                        nc.tensor.matmul(
                            pz[:, 17:18, ii], lhsT=zsq[:, ii, :],
                            rhs=ones_use, start=True, stop=True)
                    # apply: zb = r*(pz[h] - mu*S) + beff  (bf16 tail ops)
                    mu = pz[:, 16, :]
                    ssn = pz[:, 17, :]
                    mu2 = zap.tile([P, IG], F32, tag="mu2")
                    nc.scalar.activation(out=mu2, in_=mu, func=AF.Square)
                    var = zap.tile([P, IG], F32, tag="var")
                    nc.vector.tensor_tensor(var, ssn, mu2, OP.subtract)
                    r_t = zap.tile([P, IG], F32, tag="r")
                    nc.scalar.activation(out=r_t, in_=var, func=AF.Sqrt,
                                         bias=eps_t)
                    r_bf = zap.tile([P, IG], BF16, tag="rbf")
                    with nc.allow_low_precision(reason="zb is bf16 anyway"):
                        nc.vector.reciprocal(r_bf, r_t)
                    musS = zap.tile([P, H, IG], F32, tag="musS")
                    nc.vector.tensor_tensor(
                        musS, srow_bc[:, :, None].to_broadcast([P, H, IG]),
                        mu[:, None, :].to_broadcast([P, H, IG]), OP.mult)
                    zd = zap.tile([P, H, IG], BF16, tag="zd")
                    nc.vector.tensor_tensor(zd, pz[:, 0:16, :], musS,
                                            OP.subtract)
                    nc.vector.tensor_tensor(
                        zd, zd, r_bf[:, None, :].to_broadcast([P, H, IG]),
                        OP.mult)
                    nc.vector.tensor_tensor(
                        zb_all[:, jc, :, ig * IG:(ig + 1) * IG], zd,
                        beff_sb[:, jc, :, None].to_broadcast([P, H, IG]),
                        OP.add)
                run_blocks(sched[jc])

        # ---------------- phase 3: attention (S^T, j-major) ----------------
        with (
            tc.tile_pool(name="sp", bufs=2) as sp,
            tc.tile_pool(name="scps", bufs=4, space="PSUM") as scps,
            tc.tile_pool(name="ops", bufs=2, space="PSUM") as opsp,
        ):
            for h in range(H):
                hb, bb = (h % 2) * 64, h // 2
                expT = sp.tile([P, 8, P], BF16, tag="expT")
                for jq in range(2):
                    sps4 = scps.tile([P, 4, P], F32, tag="sc")
                    for q in range(4):
                        jc = jq * 4 + q
                        nc.tensor.matmul(
                            sps4[:, q, :],
                            lhsT=kT[hb:hb + HD, bb, jc * P:(jc + 1) * P],
                            rhs=qT[hb:hb + HD, bb, :],
                            start=True, stop=True)
                    nc.vector.tensor_tensor(
                        sps4, sps4, zb_all[:, jq * 4:(jq + 1) * 4, h, :],
                        OP.add)
                    nc.scalar.activation(
                        out=expT[:, jq * 4:(jq + 1) * 4, :], in_=sps4,
                        func=AF.Exp)
                o_ps = opsp.tile([P, HD + 1], F32, tag="o")
                for jc in range(8):
                    nc.tensor.matmul(
                        o_ps, lhsT=expT[:, jc, :],
                        rhs=v_aug[:, jc, h, :],
                        start=(jc == 0), stop=(jc == 7))
                rden = sp.tile([P, 1], F32, tag="rden")
                nc.vector.reciprocal(rden, o_ps[:, HD:HD + 1])
                nc.vector.tensor_scalar_mul(
                    o_sb[:, h * HD:(h + 1) * HD], o_ps[:, 0:HD], rden)

        # ---------------- phase 4: gate + output projection ----------------
        with (
            tc.tile_pool(name="fp", bufs=2) as fpool,
            tc.tile_pool(name="fps", bufs=2, space="PSUM") as fps,
        ):
            wo_sb = fpool.tile([P, CC, C], BF16)
            nc.sync.dma_start(wo_sb, wo_d[:].rearrange("(cc p) o -> p cc o", p=P))
            nc.vector.tensor_tensor(o_sb, o_sb, g_sb, OP.mult)
            goT = fpool.tile([P, CC, P], BF16)
            for cc in range(CC):
                tps = fps.tile([P, P], BF16, tag="tr2")
                gob = fpool.tile([P, P], BF16, tag="gob")
                nc.any.tensor_copy(out=gob, in_=o_sb[:, cc * P:(cc + 1) * P])
                nc.tensor.transpose(tps, gob, ident_bf)
                nc.any.tensor_copy(out=goT[:, cc, :], in_=tps)
            out_sb = fpool.tile([P, C], F32)
            for nh, w in ((0, 512), (1, 256)):
                f_full = fps.tile([P, 512], F32, tag="f", name="f_full")
                f_ps = f_full[:, :w]
                for cc in range(CC):
                    nc.tensor.matmul(
                        f_ps,
                        lhsT=goT[:, cc, :],
                        rhs=wo_sb[:, cc, nh * 512:nh * 512 + w],
                        start=(cc == 0), stop=(cc == CC - 1))
                nc.any.tensor_copy(out=out_sb[:, nh * 512:nh * 512 + w], in_=f_ps)
            nc.sync.dma_start(out_d[:], out_sb)

    nc.compile()
    return nc


_NC_CACHE = None


def kernel(s, z, mask, ln_s_w, ln_s_b, Wq, bq, Wk, Wv, Wg, ln_z_w, ln_z_b,
           Wz, Wo):
    global _NC_CACHE
    B = s.shape[0]
    s2 = np.ascontiguousarray(np.asarray(s, np.float32).reshape(N, C))
    z4 = np.asarray(z, np.float32).reshape(N, N, CZ)
    mask1 = np.asarray(mask, np.float32).reshape(N)
    wsw = np.asarray(ln_s_w, np.float32)
    wsb = np.asarray(ln_s_b, np.float32)
    Wq_, Wk_, Wv_, Wg_ = (np.asarray(w, np.float32) for w in (Wq, Wk, Wv, Wg))
    Wo_ = np.asarray(Wo, np.float32)
    sc = np.float32(1.0 / np.sqrt(HD))
    wqf = (Wq_ * wsw[None, :]) * sc
    bqf = (np.asarray(bq, np.float32) + Wq_ @ wsb) * sc
    wkf = Wk_ * wsw[None, :]
    bkf = Wk_ @ wsb
    wvf = Wv_ * wsw[None, :]
    bvf = Wv_ @ wsb
    wgf = Wg_ * wsw[None, :]
    bgf = Wg_ @ wsb
    Wz_ = np.asarray(Wz, np.float32) * np.asarray(ln_z_w, np.float32)[None, :]
    S_ = Wz_.sum(1)
    Bz = Wz_ @ np.asarray(ln_z_b, np.float32)
    beff = (Bz[None, :] + ((1.0 - mask1) * np.float32(-1e6))[:, None])
    # [N, H] -> [jn, jc, h]
    beffT = np.ascontiguousarray(
        beff.astype(np.float32).reshape(8, P, H).transpose(1, 0, 2))
    wz_aug = np.concatenate(
        [Wz_.T, np.full((CZ, 1), 1.0 / CZ, np.float32)], axis=1)

    def pad_heads(w):   # [768(o), c] -> [1024(o-padded), c]
        wp = np.zeros((1024, w.shape[1]), np.float32)
        for h in range(H):
            wp[h * 64:h * 64 + HD] = w[h * HD:(h + 1) * HD]
        return wp

    def pad_bias(b):
        bp = np.zeros(1024, np.float32)
        for h in range(H):
            bp[h * 64:h * 64 + HD] = b[h * HD:(h + 1) * HD]
        return bp

    BD = ml_dtypes.bfloat16
    common = {
        "s": s2,
        "wq": np.ascontiguousarray(pad_heads(wqf).T.astype(BD)),
        "wk": np.ascontiguousarray(pad_heads(wkf).T.astype(BD)),
        "wv": np.ascontiguousarray(wvf.T.astype(BD)),
        "wg": np.ascontiguousarray(wgf.T.astype(BD)),
        "wo": np.ascontiguousarray(Wo_.T.astype(BD)),
        "bq": np.ascontiguousarray(pad_bias(bqf).reshape(8, P)),
        "bk": np.ascontiguousarray(pad_bias(bkf).reshape(8, P)),
        "bv": np.ascontiguousarray(bvf),
        "bg": np.ascontiguousarray(bgf),
        "wz": np.ascontiguousarray(wz_aug.astype(BD)),
        "srow": np.ascontiguousarray(S_.astype(BD)),
        "beff": beffT.astype(BD),
    }
    z_bf = z4.astype(BD)
    in_maps = []
    for core in range(8):
        zs = z_bf[core * NI:(core + 1) * NI]          # [i, j, c]
        # [i, jc, jn, c] -> [c, jc, i, jn]
        zt = np.ascontiguousarray(
            zs.reshape(NI, 8, P, CZ).transpose(3, 1, 0, 2))
        m = dict(common)
        m["zt"] = zt
        m["smy"] = np.ascontiguousarray(s2[core * NI:(core + 1) * NI])
        in_maps.append(m)

    if _NC_CACHE is None:
        _NC_CACHE = build_kernel()
    import os
    trace = bool(os.environ.get("KERNEL_TRACE"))
    res = run_bass_kernel_spmd(_NC_CACHE, in_maps, core_ids=list(range(8)),
                               trace=trace)
    if res.exec_time_ns is not None:
        print(f"HW exec time: {res.exec_time_ns} ns")
        if res.instructions_and_trace is not None:
            print("trace:", res.instructions_and_trace[1])
    globals()["_LAST_RES"] = res
    out = np.concatenate([res.results[c]["out"] for c in range(8)], axis=0)
    return np.ascontiguousarray(out.reshape(B, N, C).astype(np.float32))
